# revision 80
# baseline (speedup 1.0000x reference)
"""Trainium2 Bass kernel for nn_AttentionSubLayer (dense transformer attention
sublayer with time-lerp K/V mixing, QK-norm, RoPE, GQA, per-head l2 output
norm, gating, out-proj + final RMS norm).

Sharding: 8 cores = 2 batch groups x 4-way sequence parallel with causal
load balancing.  Core c handles batch c//4 and query blocks {p, 7-p}
(256 tokens each, p = c%4).  K/V projections are computed on the owning
quarter of the sequence and AllGathered within each 4-core batch group.

v3: all matmuls bf16 (fp32 PSUM); host-side pre-transposed activations;
multiplicative 0/1 bf16 masks after exp with 1/sqrt(HD) folded into
q-hat; rsqrt Ln+Exp chains batched per stream so the scalar LUT stays on
Exp through attention; per-head l2 deferred to one epilogue via one-hot
matmul column sums.  Emission order keeps the in-order PE queue stall
free: K postproc runs under the V projection, q postproc under the G
projection, and the K/V AllGathers are split and launched as soon as each
stream is staged.  Attention processes both q-blocks at once (512-moving
scores and AV for the shared first four K blocks), rms row-sums ride the
scalar engine's Square accumulator, and rope/mask/gating work is split
between the vector and gpsimd engines.
"""

import math
import sys
import types
from contextlib import ExitStack

sys.path.insert(0, "/opt/trn_rl_repo")

import numpy as np

# ---------------------------------------------------------------- problem dims
B, T, D, H, KVH, HD = 2, 2048, 2048, 16, 4, 128
N_LAYER = 24
EPS = 1e-8
NCORE = 8
TB = 256          # token block for attention tiling
NBLK = T // TB    # 8 blocks per batch
QTOK = 2 * TB     # 512 q tokens per core
KVTOK = 2 * TB    # 512 kv tokens per core (contiguous quarter)
INV_SQRT_HD = 1.0 / math.sqrt(HD)
OUT_SCALE = 2 * N_LAYER  # final rms divided by sqrt(2*N_LAYER)


def _install_ntff_hook():
    try:
        import antenv
        if "antenv.axon_hooks" in sys.modules:
            return
        from trn_agent_boot.trn_boot import _ntff_profile_via_ctypes
        hook = _ntff_profile_via_ctypes("/opt/axon/libaxon_pjrt.so")
        mod = types.ModuleType("antenv.axon_hooks")
        mod.get_axon_ntff_profile_hook = lambda: hook
        antenv.axon_hooks = mod
        sys.modules["antenv.axon_hooks"] = mod
    except Exception:
        pass


_CACHE = {}


def _build():
    import os
    phases = os.environ.get("KERN_PHASES", "1234")
    key = ("nc", phases)
    if key in _CACHE:
        return _CACHE[key]

    import concourse.bass as bass
    import concourse.mybir as mybir
    import concourse.tile as tile
    from concourse import bacc
    from concourse.masks import make_identity

    f32 = mybir.dt.float32
    bf16 = mybir.dt.bfloat16
    AF = mybir.ActivationFunctionType
    ALU = mybir.AluOpType

    def bc_free(ap, n, at):
        """Insert a broadcast (stride-0) free dim of size n at position `at`
        of the AP's dim list (position counted incl. partition dim 0)."""
        new = list(list(d) for d in ap.ap)
        new.insert(at, [0, n])
        return bass.AP(tensor=ap.tensor, offset=ap.offset, ap=new)

    nc = bacc.Bacc("TRN2", target_bir_lowering=False, debug=False,
                   num_devices=NCORE)

    # ------------------------------------------------------------- I/O tensors
    xqT = nc.dram_tensor("xqT", [D, QTOK], bf16, kind="ExternalInput")
    xkT = nc.dram_tensor("xkT", [D, KVTOK + 128], bf16, kind="ExternalInput")
    xvT = nc.dram_tensor("xvT", [D, KVTOK + 128], bf16, kind="ExternalInput")
    Wq = nc.dram_tensor("Wq", [D, H * HD], bf16, kind="ExternalInput")
    Wg = nc.dram_tensor("Wg", [D, H * HD], bf16, kind="ExternalInput")
    Wo = nc.dram_tensor("Wo", [H * HD, D], bf16, kind="ExternalInput")
    Wkk = nc.dram_tensor("Wkk", [D, 2 * KVH * HD], bf16, kind="ExternalInput")
    Wvv = nc.dram_tensor("Wvv", [D, 2 * KVH * HD], bf16, kind="ExternalInput")
    cos_q = nc.dram_tensor("cos_q", [QTOK, HD], f32, kind="ExternalInput")
    sin_q = nc.dram_tensor("sin_q", [QTOK, HD], f32, kind="ExternalInput")
    cos_k = nc.dram_tensor("cos_k", [KVTOK, HD], f32, kind="ExternalInput")
    sin_k = nc.dram_tensor("sin_k", [KVTOK, HD], f32, kind="ExternalInput")
    # masks: one [128, 2*TB] 0/1 tile per k-block i (jq0 half of the big
    # tiles for i<4, the full small tile for i>=4; the jq1 half of big
    # tiles is always past/valid and needs no mask)
    mask_all = nc.dram_tensor("mask_all", [128, NBLK * 2 * TB], bf16,
                              kind="ExternalInput")
    ohr_h = nc.dram_tensor("ohr_h", [H, H * 128], bf16, kind="ExternalInput")
    out_y = nc.dram_tensor("out_y", [QTOK, D], f32, kind="ExternalOutput")

    # staging for K/V allgather (within 4-core batch group)
    SHARD = KVH * HD * KVTOK
    k_loc = nc.dram_tensor("k_loc", [SHARD], bf16)
    v_loc = nc.dram_tensor("v_loc", [SHARD], bf16)
    k_gath = nc.dram_tensor("k_gath", [4, SHARD], bf16)
    v_gath = nc.dram_tensor("v_gath", [4, SHARD], bf16)
    # k staged [kv, hd, t] (viewed [hd, kv, t] for the transposed store);
    # v staged [t, kv, hd]
    k_loc_T = k_loc.rearrange("(kv hd t) -> hd kv t", kv=KVH, hd=HD)
    v_loc_v = v_loc.rearrange("(t kv hd) -> t kv hd", kv=KVH, hd=HD)

    with tile.TileContext(nc) as tc, ExitStack() as es:
        # ------------------------------------------------------------ constants
        cpool = es.enter_context(tc.tile_pool(name="consts", bufs=1))
        ident = cpool.tile([128, 128], f32)
        make_identity(nc, ident[:])
        ident_bf = cpool.tile([128, 128], bf16)
        nc.vector.tensor_copy(out=ident_bf[:], in_=ident[:])
        eps_t = cpool.tile([128, 1], f32)
        nc.vector.memset(eps_t[:], EPS)
        oeps_t = cpool.tile([128, 1], f32)
        nc.vector.memset(oeps_t[:], float(OUT_SCALE) * EPS)
        lnc_t = cpool.tile([128, 1], f32)
        nc.vector.memset(lnc_t[:], math.log(INV_SQRT_HD))
        # one-hot column tiles: oh_cols[:, h, :] has column h all-ones
        oh_cols = cpool.tile([128, H, H], bf16)
        nc.vector.memset(oh_cols[:], 0.0)
        for h in range(H):
            nc.vector.memset(oh_cols[:, h, h:h + 1], 1.0)
        # one-hot row tiles: ohr[:, 128h:128h+128] has row h all-ones
        ohr = cpool.tile([H, H * 128], bf16)
        nc.sync.dma_start(out=ohr[:], in_=ohr_h[:])
        cosq_sb = cpool.tile([128, 4, HD], f32)
        sinq_sb = cpool.tile([128, 4, HD], f32)
        cosk_sb = cpool.tile([128, 4, HD], f32)
        sink_sb = cpool.tile([128, 4, HD], f32)
        nc.sync.dma_start(out=cosq_sb[:], in_=cos_q.rearrange("(m p) d -> p m d", p=128))
        nc.sync.dma_start(out=sinq_sb[:], in_=sin_q.rearrange("(m p) d -> p m d", p=128))
        nc.sync.dma_start(out=cosk_sb[:], in_=cos_k.rearrange("(m p) d -> p m d", p=128))
        nc.sync.dma_start(out=sink_sb[:], in_=sin_k.rearrange("(m p) d -> p m d", p=128))

        # ============================================================ helpers
        def rms_sumsq(x_t, nh, s2, scrap):
            """s2[:, h] = sum over HD of x_t[:, h*128:...]^2 via the scalar
            engine's Square + row-accumulator (Square lives in every LUT set,
            so no table reload)."""
            for h in range(nh):
                nc.scalar.activation(out=scrap[:], in_=x_t[:, 128 * h:128 * h + 128],
                                     func=AF.Square, accum_out=s2[:, h:h + 1])

        def rms_apply(x_t, nh, ri):
            """x_t *= ri per head (broadcast over HD)."""
            x3 = x_t[:].rearrange("p (h d) -> p h d", h=nh)
            ri_b = bc_free(ri, 128, 2)
            nc.vector.tensor_tensor(out=x3, in0=x3, in1=ri_b, op=ALU.mult)

        def rope_to_bf(dst_bf, src, nh, cos_sb, sin_sb, m, t1, t2):
            """dst_bf bf16 [128, nh*HD] = rope(src f32), ops split between the
            vector (cos mult + lo half) and gpsimd (hi half) engines."""
            half = HD // 2
            d3 = dst_bf[:].rearrange("p (h d) -> p h d", h=nh)
            s3 = src[:].rearrange("p (h d) -> p h d", h=nh)
            cos_b = bc_free(cos_sb[:, m, :], nh, 1)
            sin_lo = bc_free(sin_sb[:, m, 0:half], nh, 1)
            sin_hi = bc_free(sin_sb[:, m, half:HD], nh, 1)
            nc.vector.tensor_tensor(out=d3, in0=s3, in1=cos_b, op=ALU.mult)
            nc.vector.tensor_tensor(out=t1[:], in0=s3[:, :, half:HD],
                                    in1=sin_lo, op=ALU.mult)
            nc.vector.tensor_tensor(out=d3[:, :, 0:half], in0=d3[:, :, 0:half],
                                    in1=t1[:], op=ALU.subtract)
            nc.gpsimd.tensor_tensor(out=t2[:], in0=s3[:, :, 0:half],
                                    in1=sin_hi, op=ALU.mult)
            nc.gpsimd.tensor_tensor(out=d3[:, :, half:HD], in0=d3[:, :, half:HD],
                                    in1=t2[:], op=ALU.add)

        # ===================================================== phase 1: K / V
        k_stage, v_stage = [], []
        with tc.tile_pool(name="p1xt", bufs=1) as xtp, \
             tc.tile_pool(name="p1w", bufs=1) as wp, \
             tc.tile_pool(name="p1kv", bufs=1) as kvp, \
             tc.tile_pool(name="p1ps", bufs=1, space="PSUM") as pskv, \
             tc.tile_pool(name="p1pt", bufs=2, space="PSUM") as ptp, \
             tc.tile_pool(name="p1sm", bufs=2) as smp:
            xkT_sb = xtp.tile([128, 16, KVTOK + 128], bf16, name="xkT_sb")
            xvT_sb = xtp.tile([128, 16, KVTOK + 128], bf16, name="xvT_sb")
            wk_t = [wp.tile([128, 2 * KVH * HD], bf16, tag=f"w{k}",
                            name=f"wk{k}") for k in range(16)]
            wv_t = [wp.tile([128, 2 * KVH * HD], bf16, tag=f"wv{k}",
                            name=f"wv{k}") for k in range(16)]
            # DMA issue order = transfer order: each stream's weights land
            # right after its activations so the first matmuls start early
            nc.sync.dma_start(out=xkT_sb[:],
                              in_=xkT.rearrange("(k p) t -> p k t", p=128))
            for k in range(16):
                nc.sync.dma_start(out=wk_t[k][:], in_=Wkk[128 * k:128 * k + 128, :])
            nc.sync.dma_start(out=xvT_sb[:],
                              in_=xvT.rearrange("(k p) t -> p k t", p=128))
            for k in range(16):
                nc.sync.dma_start(out=wv_t[k][:], in_=Wvv[128 * k:128 * k + 128, :])
            s2k = kvp.tile([128, 16], f32, name="s2k")
            s2v = kvp.tile([128, 16], f32, name="s2v")
            rik = kvp.tile([128, 16], f32, name="rik")
            riv = kvp.tile([128, 16], f32, name="riv")
            sq_scrap = kvp.tile([128, HD], f32, name="sqsc")
            nat = {}

            pskv_t = {}

            def kv_proj(xT_sb, wts, stg, k0, k1):
                if (stg, 0) not in pskv_t:
                    pskv_t[stg, 0] = [
                        pskv.tile([128, KVH * HD], f32, tag=f"pkv{m}",
                                  name=f"pkv{stg}{m}") for m in range(4)]
                ps = pskv_t[stg, 0]
                for k in range(k0, k1):
                    wt = wts[k]
                    for m in range(4):
                        nc.tensor.matmul(ps[m][:],
                                         xT_sb[:, k, 128 + 128 * m:256 + 128 * m],
                                         wt[:, :KVH * HD], start=(k == 0), stop=False)
                        nc.tensor.matmul(ps[m][:],
                                         xT_sb[:, k, 127 + 128 * m:255 + 128 * m],
                                         wt[:, KVH * HD:], start=False, stop=(k == 15))
                if k1 == 16:
                    for m in range(4):
                        t = kvp.tile([128, KVH * HD], f32, name=f"nat{stg}{m}")
                        nat[stg, m] = t
                        nc.scalar.copy(out=t[:], in_=ps[m][:])

            def rsqrt_batch(s2, ri, bias):
                ln = smp.tile([128, 16], f32, tag="ln")
                nc.scalar.activation(out=ln[:], in_=s2[:], func=AF.Ln,
                                     bias=eps_t[:], scale=1.0 / HD)
                if bias is None:
                    nc.scalar.activation(out=ri, in_=ln[:], func=AF.Exp, scale=-0.5)
                else:
                    nc.scalar.activation(out=ri, in_=ln[:], func=AF.Exp,
                                         scale=-0.5, bias=bias)

            # K projection, K row-sums + rsqrt (scalar runs under V proj)
            kv_proj(xkT_sb, wk_t, "k", 0, 16)
            for m in range(4):
                rms_sumsq(nat["k", m], KVH, s2k[:, 4 * m:4 * m + 4], sq_scrap)
            rsqrt_batch(s2k[:], rik[:], None)
            # first half of the V projection keeps the PE busy under the K
            # postproc chain; K transposes then slot in with zero PE stall
            kv_proj(xvT_sb, wv_t, "v", 0, 8)
            # K scale + rope + transpose + stage -> AllGather(K)
            for m in range(4):
                t = nat["k", m]
                rms_apply(t, KVH, rik[:, 4 * m:4 * m + 4])
                rot_bf = smp.tile([128, KVH * HD], bf16, tag="rotbf")
                t1 = smp.tile([128, KVH, HD // 2], f32, tag="t1")
                t2 = smp.tile([128, KVH, HD // 2], f32, tag="t2")
                rope_to_bf(rot_bf, t, KVH, cosk_sb, sink_sb, m, t1, t2)
                kst = smp.tile([128, KVH, 128], bf16, tag="kst")
                for kv in range(KVH):
                    pst = ptp.tile([128, 128], bf16, tag="pst")
                    nc.tensor.transpose(pst[:], rot_bf[:, 128 * kv:128 * kv + 128],
                                        ident_bf[:])
                    nc.scalar.copy(out=kst[:, kv, :], in_=pst[:])
                d = nc.scalar.dma_start(
                    out=k_loc_T[:, :, 128 * m:128 * m + 128], in_=kst[:])
                k_stage.append(d)
            ag_k = nc.gpsimd.collective_compute(
                "AllGather", ALU.bypass,
                replica_groups=[[0, 1, 2, 3], [4, 5, 6, 7]],
                ins=[k_loc[:]], outs=[k_gath[:]])
            for d in k_stage:
                tile.add_dep_helper(ag_k.ins, d.ins, reason="k stage before ag")
            # second half of the V projection
            kv_proj(xvT_sb, wv_t, "v", 8, 16)
            # V row-sums + rsqrt + scale (writes bf16) + stage -> AllGather(V)
            for m in range(4):
                rms_sumsq(nat["v", m], KVH, s2v[:, 4 * m:4 * m + 4], sq_scrap)
            rsqrt_batch(s2v[:], riv[:], None)
            for m in range(4):
                t = nat["v", m]
                vr = smp.tile([128, KVH * HD], bf16, tag="vr")
                v3 = vr[:].rearrange("p (h d) -> p h d", h=KVH)
                t3 = t[:].rearrange("p (h d) -> p h d", h=KVH)
                ri_b = bc_free(riv[:, 4 * m:4 * m + 4], 128, 2)
                nc.vector.tensor_tensor(out=v3, in0=t3, in1=ri_b, op=ALU.mult)
                d = nc.scalar.dma_start(
                    out=v_loc_v[128 * m:128 * m + 128, :, :],
                    in_=vr[:].rearrange("p (h d) -> p h d", h=KVH))
                v_stage.append(d)
            ag_v = nc.gpsimd.collective_compute(
                "AllGather", ALU.bypass,
                replica_groups=[[0, 1, 2, 3], [4, 5, 6, 7]],
                ins=[v_loc[:]], outs=[v_gath[:]])
            for d in v_stage:
                tile.add_dep_helper(ag_v.ins, d.ins, reason="v stage before ag")

        if "2" not in phases:
            with tc.tile_pool(name="dbg1", bufs=1) as dbp:
                for m in range(4):
                    t = dbp.tile([128, D], f32, tag="dbg")
                    nc.vector.memset(t[:], 0.0)
                    nc.sync.dma_start(out=out_y[128 * m:128 * m + 128, :], in_=t[:])

        # persistent pools (opened after phase-1 pools are released; the xq
        # pool opens last so it can be released again after phase 2)
        p_gT = es.enter_context(tc.tile_pool(name="ppgT", bufs=1))
        gT_sb = p_gT.tile([128, H, QTOK], bf16, name="gT_sb")
        p_qT = es.enter_context(tc.tile_pool(name="ppqT", bufs=1))
        qT_sb = p_qT.tile([128, H, QTOK], bf16, name="qT_sb")
        p_gTr = es.enter_context(tc.tile_pool(name="ppgTr", bufs=1))
        gTr_sb = p_gTr.tile([128, H, QTOK], bf16, name="gTr_sb")
        xq_es = ExitStack()
        p_xq = xq_es.enter_context(tc.tile_pool(name="ppxq", bufs=1))
        xqT_sb = p_xq.tile([128, 16, QTOK], bf16, name="xqT_sb")

        # ===================================================== phase 2: Q / G
        nc.sync.dma_start(out=xqT_sb[:],
                          in_=xqT.rearrange("(k p) t -> p k t", p=128))
        if "2" in phases:
          with tc.tile_pool(name="p2w", bufs=1) as wp, \
               tc.tile_pool(name="p2q", bufs=1) as qp, \
               tc.tile_pool(name="p2ps", bufs=1, space="PSUM") as psq, \
               tc.tile_pool(name="p2pt", bufs=2, space="PSUM") as ptp, \
               tc.tile_pool(name="p2sm", bufs=2) as smp:
            # full-row weight tiles: 16 DMA issues per matrix instead of 64
            wq_t = [wp.tile([128, H * HD], bf16, tag=f"w{k}", name=f"wq{k}")
                    for k in range(16)]
            for k in range(16):
                nc.sync.dma_start(out=wq_t[k][:], in_=Wq[128 * k:128 * k + 128, :])

            # Q projection -> natural [tok, H*HD]
            q_sb = [qp.tile([128, H * HD], f32, name=f"q{m}") for m in range(4)]
            for n in range(4):
                ps = [psq.tile([128, 512], f32, tag=f"pp{m}", name=f"pq{m}")
                      for m in range(4)]
                for k in range(16):
                    for m in range(4):
                        nc.tensor.matmul(ps[m][:],
                                         xqT_sb[:, k, 128 * m:128 * m + 128],
                                         wq_t[k][:, 512 * n:512 * n + 512],
                                         start=(k == 0), stop=(k == 15))
                for m in range(4):
                    nc.scalar.copy(out=q_sb[m][:, 512 * n:512 * n + 512], in_=ps[m][:])

            # q row-sums + rsqrt (scale folds 1/sqrt(HD)); runs under G proj
            s2q = qp.tile([128, 4, H], f32, name="s2q")
            riq = qp.tile([128, 4, H], f32, name="riq")
            sq_scrap = qp.tile([128, HD], f32, name="sqscq")
            for m in range(4):
                rms_sumsq(q_sb[m], H, s2q[:, m, :], sq_scrap)
            for m in range(4):
                ln = smp.tile([128, H], f32, tag="qln")
                nc.scalar.activation(out=ln[:], in_=s2q[:, m, :], func=AF.Ln,
                                     bias=eps_t[:], scale=1.0 / HD)
                nc.scalar.activation(out=riq[:, m, :], in_=ln[:], func=AF.Exp,
                                     scale=-0.5, bias=lnc_t[:])

            # G projection -> transposed [gcol, tok] directly, bf16
            wg_t = [wp.tile([128, H * HD], bf16, tag=f"w{k}", name=f"wg{k}")
                    for k in range(16)]
            for k in range(16):
                nc.sync.dma_start(out=wg_t[k][:], in_=Wg[128 * k:128 * k + 128, :])
            for gq in range(4):
                psg = [psq.tile([128, 512], f32, tag=f"pp{i}", name=f"pg{i}")
                       for i in range(4)]
                for k in range(16):
                    for gi in range(4):
                        nc.tensor.matmul(
                            psg[gi][:],
                            wg_t[k][:, 512 * gq + 128 * gi:512 * gq + 128 * gi + 128],
                            xqT_sb[:, k, :],
                            start=(k == 0), stop=(k == 15))
                for gi in range(4):
                    nc.scalar.copy(out=gT_sb[:, 4 * gq + gi, :], in_=psg[gi][:])

            # q scale + rope (under G proj) then transpose
            rots = []
            for m in range(4):
                rms_apply(q_sb[m], H, riq[:, m, :])
                rot_bf = smp.tile([128, H * HD], bf16, tag="qrotbf",
                                  name=f"qrot{m}")
                t1 = smp.tile([128, H, HD // 2], f32, tag="qt1")
                t2 = smp.tile([128, H, HD // 2], f32, tag="qt2")
                rope_to_bf(rot_bf, q_sb[m], H, cosq_sb, sinq_sb, m, t1, t2)
                rots.append(rot_bf)
            for m in range(4):
                for h in range(H):
                    pst = ptp.tile([128, 128], bf16, tag="pst")
                    nc.tensor.transpose(pst[:], rots[m][:, 128 * h:128 * h + 128],
                                        ident_bf[:])
                    nc.scalar.copy(out=qT_sb[:, h, 128 * m:128 * m + 128], in_=pst[:])

        if "2" in phases and "3" not in phases:
            with tc.tile_pool(name="dbg2", bufs=1) as dbp:
                for m in range(4):
                    t = dbp.tile([128, D], f32, tag="dbg")
                    nc.vector.tensor_copy(
                        out=t[:],
                        in_=gT_sb[:, 4 * m:4 * m + 4, :].rearrange("p a b -> p (a b)"))
                    nc.sync.dma_start(out=out_y[128 * m:128 * m + 128, :], in_=t[:])

        xq_es.close()
        # ==================================================== phase 3: attention
        # out-proj weights prefetched during attention so phase 4 never
        # waits on SBUF freed by attention tiles
        p_wo = es.enter_context(tc.tile_pool(name="ppwo", bufs=1))
        wo_t = [p_wo.tile([128, D], bf16, tag=f"wo{k}", name=f"wo{k}")
                for k in range(16)]
        if "4" in phases:
            for k in range(16):
                nc.sync.dma_start(out=wo_t[k][:], in_=Wo[128 * k:128 * k + 128, :])
        if "3" in phases:
          with tc.tile_pool(name="p3m", bufs=1) as mp, \
               tc.tile_pool(name="p3kv", bufs=1) as kvp, \
               tc.tile_pool(name="p3pt", bufs=3) as ptq, \
               tc.tile_pool(name="p3y", bufs=1) as yp, \
               tc.tile_pool(name="p3py", bufs=2, space="PSUM") as psy_p, \
               tc.tile_pool(name="p3pn", bufs=1, space="PSUM") as psn_p, \
               tc.tile_pool(name="p3sm", bufs=2) as smp:
            masks_sb = mp.tile([128, NBLK, 2, TB], bf16, name="masks")
            nc.sync.dma_start(
                out=masks_sb[:],
                in_=mask_all.rearrange("p (i s t) -> p i s t", i=NBLK, s=2))

            # gathered K: [128(hd), kv, shard, t] ; V: [128(tok%128), g, kv, hd]
            K_all = kvp.tile([128, KVH, 4, KVTOK], bf16, name="K_all")
            V_all = kvp.tile([128, 16, KVH, HD], bf16, name="V_all")
            for sh in range(4):
                kg = k_gath[sh].rearrange("(kv hd t) -> kv hd t", kv=KVH, hd=HD)
                vg = v_gath[sh].rearrange("(t kv hd) -> t kv hd", kv=KVH, hd=HD)
                d = nc.sync.dma_start(out=K_all[:, :, sh, :],
                                      in_=kg.rearrange("kv d t -> d kv t"))
                tile.add_dep_helper(d.ins, ag_k.ins, reason="ag before k load")
                d = nc.sync.dma_start(
                    out=V_all[:, 4 * sh:4 * sh + 4, :, :],
                    in_=vg.rearrange("(a p) kv d -> p a kv d", p=128))
                tile.add_dep_helper(d.ins, ag_v.ins, reason="ag before v load")

            y_sb = yp.tile([128, H, QTOK], bf16, name="y_sb")
            n2_ps = psn_p.tile([H, 2 * TB], f32, name="n2")
            # i-order puts full-region AV matmuls at the start and stop flags
            IORD = [0, 4, 5, 6, 7, 1, 2, 3]
            pss_es = ExitStack()
            pss_p = pss_es.enter_context(
                tc.tile_pool(name="p3ps", bufs=2, space="PSUM"))
            for h in range(H):
                kv = h // 4
                psy = psy_p.tile([128, 2 * TB], f32, tag="psy")
                pts = []
                for step in range(len(IORD) + 1):
                    if step < len(IORD):
                        i = IORD[step]
                        big = i < 4
                        if big:
                            pss = pss_p.tile([128, 2, 2 * TB], f32, tag="pss")
                            qs = qT_sb[:, h, :]
                        else:
                            pss = pss_p.tile([128, 2, TB], f32, tag="pss")
                            qs = qT_sb[:, h, TB:2 * TB]
                        for ss in range(2):
                            nc.tensor.matmul(
                                pss[:, ss, :],
                                K_all[:, kv, i // 2,
                                      TB * (i % 2) + 128 * ss:
                                      TB * (i % 2) + 128 * ss + 128],
                                qs, start=True, stop=True)
                        w = 2 * TB if big else TB
                        pt = ptq.tile([128, 2, w], bf16, tag="pt")
                        nc.scalar.activation(
                            out=pt[:].rearrange("p a b -> p (a b)"),
                            in_=pss[:].rearrange("p a b -> p (a b)"), func=AF.Exp)
                        # big tiles: mask only the jq0 half (jq1 half of the
                        # first 4 k-blocks is always past/valid)
                        nc.vector.tensor_tensor(
                            out=pt[:, :, 0:TB], in0=pt[:, :, 0:TB],
                            in1=masks_sb[:, i, :, :], op=ALU.mult)
                        pts.append((i, big, pt))
                    if step >= 1:
                        i, big, pt = pts[step - 1]
                        for ss in range(2):
                            if big:
                                nc.tensor.matmul(
                                    psy[:], V_all[:, 2 * i + ss, kv, :],
                                    pt[:, ss, :],
                                    start=(step == 1 and ss == 0),
                                    stop=(step == len(IORD) and ss == 1))
                            else:
                                nc.tensor.matmul(
                                    psy[:, TB:2 * TB],
                                    V_all[:, 2 * i + ss, kv, :],
                                    pt[:, ss, :], start=False, stop=False)
                ysq = smp.tile([128, 2 * TB], bf16, tag="ysq")
                nc.vector.tensor_copy(out=y_sb[:, h, :], in_=psy[:])
                nc.vector.tensor_tensor(out=ysq[:], in0=y_sb[:, h, :],
                                        in1=y_sb[:, h, :], op=ALU.mult)
                nc.tensor.matmul(n2_ps[:], oh_cols[:, h, :], ysq[:],
                                 start=(h == 0), stop=(h == H - 1))
            pss_es.close()
            psb_p = pss_es.enter_context(
                tc.tile_pool(name="p3pb", bufs=2, space="PSUM"))
            # epilogue: one Ln+Exp pair for all 32 l2 norms, broadcast + gate
            lnn = smp.tile([H, 2 * TB], f32, tag="lnn")
            nc.scalar.activation(out=lnn[:], in_=n2_ps[:], func=AF.Ln)
            rsq = smp.tile([H, 2 * TB], bf16, tag="rsq")
            nc.scalar.activation(out=rsq[:], in_=lnn[:], func=AF.Exp, scale=-0.5)
            for h in range(H):
                psb = psb_p.tile([128, 2 * TB], f32, tag="psb")
                nc.tensor.matmul(psb[:], ohr[:, 128 * h:128 * h + 128],
                                 rsq[:], start=True, stop=True)
                tmp = smp.tile([128, 2 * TB], f32, tag=f"ytmp{h % 2}")
                nc.gpsimd.tensor_tensor(out=tmp[:], in0=y_sb[:, h, :],
                                        in1=gT_sb[:, h, :], op=ALU.mult)
                nc.vector.tensor_tensor(out=gTr_sb[:, h, :], in0=tmp[:],
                                        in1=psb[:], op=ALU.mult)
            pss_es.close()

        if "3" in phases and "4" not in phases:
            with tc.tile_pool(name="dbg3", bufs=1) as dbp:
                for m in range(4):
                    t = dbp.tile([128, D], f32, tag="dbg")
                    nc.vector.tensor_copy(
                        out=t[:],
                        in_=gTr_sb[:, 4 * m:4 * m + 4, :].rearrange("p a b -> p (a b)"))
                    nc.sync.dma_start(out=out_y[128 * m:128 * m + 128, :], in_=t[:])

        # ==================================================== phase 4: out proj
        if "4" in phases:
          with tc.tile_pool(name="p4o", bufs=2) as op_, \
               tc.tile_pool(name="p4ps", bufs=2, space="PSUM") as pso_p, \
               tc.tile_pool(name="p4sm", bufs=2) as smp:
            # pipelined per m-tile: each 128-token tile finishes its matmuls,
            # rms and store while the next tile's matmuls run
            for m in range(4):
                pso = [pso_p.tile([128, 512], f32, tag=f"po{n}", name=f"po{n}")
                       for n in range(4)]
                for k in range(16):
                    for n in range(4):
                        nc.tensor.matmul(pso[n][:],
                                         gTr_sb[:, k, 128 * m:128 * m + 128],
                                         wo_t[k][:, 512 * n:512 * n + 512],
                                         start=(k == 0), stop=(k == 15))
                o_sb = op_.tile([128, D], f32, tag="o", name=f"o{m}")
                s2o = smp.tile([128, 1], f32, tag="s2o")
                sq_sc = smp.tile([128, D], f32, tag="osc")
                for n in range(4):
                    nc.scalar.copy(out=o_sb[:, 512 * n:512 * n + 512],
                                   in_=pso[n][:])
                nc.vector.tensor_tensor(out=sq_sc[:], in0=o_sb[:],
                                        in1=o_sb[:], op=ALU.mult)
                nc.vector.tensor_reduce(out=s2o[:], in_=sq_sc[:],
                                        axis=mybir.AxisListType.X, op=ALU.add)
                lno = smp.tile([128, 1], f32, tag="lno")
                nc.scalar.activation(out=lno[:], in_=s2o[:], func=AF.Ln,
                                     bias=oeps_t[:], scale=float(OUT_SCALE) / D)
                r2o = smp.tile([128, 1], f32, tag="r2o")
                nc.scalar.activation(out=r2o[:], in_=lno[:], func=AF.Exp,
                                     scale=-0.5)
                nc.vector.tensor_scalar_mul(o_sb[:], o_sb[:], r2o[:])
                nc.sync.dma_start(out=out_y[128 * m:128 * m + 128, :],
                                  in_=o_sb[:])

    nc.compile()
    _CACHE[key] = nc
    return nc


def _host_inputs(xq, xk, xv, Wq, Wk, Wv, Wg, Wo, mix_k, mix_v):
    """Build the 8 per-core input maps (bf16 weights/activations)."""
    import ml_dtypes
    f = np.float32
    bf = ml_dtypes.bfloat16
    xq = np.asarray(xq, f)
    xk = np.asarray(xk, f)
    xv = np.asarray(xv, f)
    Wq = np.asarray(Wq, f)
    Wk = np.asarray(Wk, f)
    Wv = np.asarray(Wv, f)
    Wg = np.asarray(Wg, f)
    Wo = np.asarray(Wo, f)
    mix_k = np.asarray(mix_k, f)
    mix_v = np.asarray(mix_v, f)

    Wkk = np.ascontiguousarray(np.concatenate(
        [(1.0 - mix_k)[:, None] * Wk, mix_k[:, None] * Wk], axis=1)).astype(bf)
    Wvv = np.ascontiguousarray(np.concatenate(
        [(1.0 - mix_v)[:, None] * Wv, mix_v[:, None] * Wv], axis=1)).astype(bf)
    Wq_b = np.ascontiguousarray(Wq).astype(bf)
    Wg_b = np.ascontiguousarray(Wg).astype(bf)
    Wo_b = np.ascontiguousarray(Wo).astype(bf)

    half = HD // 2
    inv_freq = 1.0 / (10000.0 ** (np.arange(half, dtype=np.float64) / half))
    ang = np.arange(T, dtype=np.float64)[:, None] * inv_freq[None, :]
    cos_t = np.concatenate([np.cos(ang), np.cos(ang)], axis=-1).astype(f)
    sin_t = np.concatenate([np.sin(ang), np.sin(ang)], axis=-1).astype(f)

    # multiplicative post-exp masks; pt subtile ss holds tk rows
    # 128*ss..128*ss+127 of k-block i; valid iff global tk <= global tq.
    ii = np.arange(128)[:, None]
    jj = np.arange(TB)[None, :]
    diag_mask = np.empty((128, 2, TB), f)
    for ss in range(2):
        diag_mask[:, ss, :] = (128 * ss + ii <= jj).astype(f)
    ones_m = np.ones((128, 2, TB), f)
    zeros_m = np.zeros((128, 2, TB), f)

    def blk_mask(i, jq):
        return diag_mask if i == jq else (ones_m if i < jq else zeros_m)

    ohr_np = np.zeros((H, H * 128), f)
    for h in range(H):
        ohr_np[h, 128 * h:128 * h + 128] = 1.0
    ohr_np = ohr_np.astype(bf)

    in_maps = []
    for c in range(NCORE):
        b, p = divmod(c, 4)
        jq0, jq1 = p, NBLK - 1 - p
        rows_q = np.concatenate([np.arange(TB * jq0, TB * jq0 + TB),
                                 np.arange(TB * jq1, TB * jq1 + TB)])
        t0 = KVTOK * p
        rows_kv = np.arange(t0, t0 + KVTOK)

        xqT_s = np.ascontiguousarray(xq[b, rows_q, :].T.astype(bf))
        xk_s = np.zeros((KVTOK + 128, D), f)
        xv_s = np.zeros((KVTOK + 128, D), f)
        xk_s[128:] = xk[b, t0:t0 + KVTOK, :]
        xv_s[128:] = xv[b, t0:t0 + KVTOK, :]
        if p > 0:
            xk_s[127] = xk[b, t0 - 1, :]
            xv_s[127] = xv[b, t0 - 1, :]
        xkT_s = np.ascontiguousarray(xk_s.T.astype(bf))
        xvT_s = np.ascontiguousarray(xv_s.T.astype(bf))

        # one [128, ss, TB] tile per k-block: jq0 mask for i<4 (the jq1 half
        # of big tiles is always valid), jq1 mask for i>=4
        mask = np.empty((128, NBLK * 2 * TB), f)
        for i in range(NBLK):
            ms = blk_mask(i, jq0 if i < 4 else jq1).reshape(128, 2 * TB)
            mask[:, 2 * TB * i:2 * TB * (i + 1)] = ms

        in_maps.append({
            "xqT": xqT_s, "xkT": xkT_s, "xvT": xvT_s,
            "Wq": Wq_b, "Wg": Wg_b, "Wo": Wo_b,
            "Wkk": Wkk, "Wvv": Wvv,
            "cos_q": np.ascontiguousarray(cos_t[rows_q]),
            "sin_q": np.ascontiguousarray(sin_t[rows_q]),
            "cos_k": np.ascontiguousarray(cos_t[rows_kv]),
            "sin_k": np.ascontiguousarray(sin_t[rows_kv]),
            "mask_all": mask.astype(bf),
            "ohr_h": ohr_np,
        })
    return in_maps


def _run(in_maps, trace=False, tmpdir=None):
    _install_ntff_hook()
    from concourse.bass_utils import run_bass_kernel_spmd
    nc = _build()
    return run_bass_kernel_spmd(nc, in_maps, list(range(NCORE)),
                                trace=trace, tmpdir=tmpdir)


def kernel(xq, xk, xv, Wq, Wk, Wv, Wg, Wo, mix_k, mix_v,
           _trace=False, _tmpdir=None):
    in_maps = _host_inputs(xq, xk, xv, Wq, Wk, Wv, Wg, Wo, mix_k, mix_v)
    res = _run(in_maps, trace=_trace, tmpdir=_tmpdir)
    out = np.empty((B, T, D), np.float32)
    for c in range(NCORE):
        b, p = divmod(c, 4)
        jq0, jq1 = p, NBLK - 1 - p
        y = res.results[c]["out_y"]
        out[b, TB * jq0:TB * jq0 + TB, :] = y[:TB]
        out[b, TB * jq1:TB * jq1 + TB, :] = y[TB:]
    kernel._last_exec_ns = res.exec_time_ns
    return out


# revision 92
# speedup vs baseline: 1.0373x; 1.0373x over previous
"""Trainium2 Bass kernel for nn_AttentionSubLayer (dense transformer attention
sublayer with time-lerp K/V mixing, QK-norm, RoPE, GQA, per-head l2 output
norm, gating, out-proj + final RMS norm).

Sharding: 8 cores = 2 batch groups x 4-way sequence parallel with causal
load balancing.  Core c handles batch c//4 and query blocks {p, 7-p}
(256 tokens each, p = c%4).  K/V projections are computed on the owning
quarter of the sequence and AllGathered within each 4-core batch group.

v3: all matmuls bf16 (fp32 PSUM); host-side pre-transposed activations;
multiplicative 0/1 bf16 masks after exp with 1/sqrt(HD) folded into
q-hat; rsqrt Ln+Exp chains batched per stream so the scalar LUT stays on
Exp through attention; per-head l2 deferred to one epilogue via one-hot
matmul column sums.  Emission order keeps the in-order PE queue stall
free: K postproc runs under the V projection, q postproc under the G
projection, and the K/V AllGathers are split and launched as soon as each
stream is staged.  Attention processes both q-blocks at once (512-moving
scores and AV for the shared first four K blocks), rms row-sums ride the
scalar engine's Square accumulator, and rope/mask/gating work is split
between the vector and gpsimd engines.
"""

import math
import sys
import types
from contextlib import ExitStack

sys.path.insert(0, "/opt/trn_rl_repo")

import numpy as np

# ---------------------------------------------------------------- problem dims
B, T, D, H, KVH, HD = 2, 2048, 2048, 16, 4, 128
N_LAYER = 24
EPS = 1e-8
NCORE = 8
TB = 256          # token block for attention tiling
NBLK = T // TB    # 8 blocks per batch
QTOK = 2 * TB     # 512 q tokens per core
KVTOK = 2 * TB    # 512 kv tokens per core (contiguous quarter)
INV_SQRT_HD = 1.0 / math.sqrt(HD)
OUT_SCALE = 2 * N_LAYER  # final rms divided by sqrt(2*N_LAYER)


def _install_ntff_hook():
    try:
        import antenv
        if "antenv.axon_hooks" in sys.modules:
            return
        from trn_agent_boot.trn_boot import _ntff_profile_via_ctypes
        hook = _ntff_profile_via_ctypes("/opt/axon/libaxon_pjrt.so")
        mod = types.ModuleType("antenv.axon_hooks")
        mod.get_axon_ntff_profile_hook = lambda: hook
        antenv.axon_hooks = mod
        sys.modules["antenv.axon_hooks"] = mod
    except Exception:
        pass


_CACHE = {}


def _build():
    import os
    phases = os.environ.get("KERN_PHASES", "1234")
    key = ("nc", phases)
    if key in _CACHE:
        return _CACHE[key]

    import concourse.bass as bass
    import concourse.mybir as mybir
    import concourse.tile as tile
    from concourse import bacc
    from concourse.masks import make_identity

    f32 = mybir.dt.float32
    bf16 = mybir.dt.bfloat16
    AF = mybir.ActivationFunctionType
    ALU = mybir.AluOpType

    def bc_free(ap, n, at):
        """Insert a broadcast (stride-0) free dim of size n at position `at`
        of the AP's dim list (position counted incl. partition dim 0)."""
        new = list(list(d) for d in ap.ap)
        new.insert(at, [0, n])
        return bass.AP(tensor=ap.tensor, offset=ap.offset, ap=new)

    nc = bacc.Bacc("TRN2", target_bir_lowering=False, debug=False,
                   num_devices=NCORE)

    # ------------------------------------------------------------- I/O tensors
    xqT = nc.dram_tensor("xqT", [D, QTOK], bf16, kind="ExternalInput")
    xkT = nc.dram_tensor("xkT", [D, KVTOK + 128], bf16, kind="ExternalInput")
    xvT = nc.dram_tensor("xvT", [D, KVTOK + 128], bf16, kind="ExternalInput")
    Wq = nc.dram_tensor("Wq", [D, H * HD], bf16, kind="ExternalInput")
    Wg = nc.dram_tensor("Wg", [D, H * HD], bf16, kind="ExternalInput")
    Wo = nc.dram_tensor("Wo", [H * HD, D], bf16, kind="ExternalInput")
    Wkk = nc.dram_tensor("Wkk", [D, 2 * KVH * HD], bf16, kind="ExternalInput")
    Wvv = nc.dram_tensor("Wvv", [D, 2 * KVH * HD], bf16, kind="ExternalInput")
    cos_q = nc.dram_tensor("cos_q", [QTOK, HD], f32, kind="ExternalInput")
    sin_q = nc.dram_tensor("sin_q", [QTOK, HD], f32, kind="ExternalInput")
    cos_k = nc.dram_tensor("cos_k", [KVTOK, HD], f32, kind="ExternalInput")
    sin_k = nc.dram_tensor("sin_k", [KVTOK, HD], f32, kind="ExternalInput")
    # masks: one [128, 2*TB] 0/1 tile per k-block i (jq0 half of the big
    # tiles for i<4, the full small tile for i>=4; the jq1 half of big
    # tiles is always past/valid and needs no mask)
    mask_all = nc.dram_tensor("mask_all", [128, NBLK * 2 * TB], bf16,
                              kind="ExternalInput")
    ohr_h = nc.dram_tensor("ohr_h", [H, H * 128], bf16, kind="ExternalInput")
    out_y = nc.dram_tensor("out_y", [QTOK, D], f32, kind="ExternalOutput")

    # staging for K/V allgather (within 4-core batch group)
    SHARD = KVH * HD * KVTOK
    k_loc = nc.dram_tensor("k_loc", [SHARD], bf16)
    v_loc = nc.dram_tensor("v_loc", [SHARD], bf16)
    k_gath = nc.dram_tensor("k_gath", [4, SHARD], bf16)
    v_gath = nc.dram_tensor("v_gath", [4, SHARD], bf16)
    # k staged [kv, hd, t] (viewed [hd, kv, t] for the transposed store);
    # v staged [t, kv, hd]
    k_loc_T = k_loc.rearrange("(kv hd t) -> hd kv t", kv=KVH, hd=HD)
    v_loc_v = v_loc.rearrange("(t kv hd) -> t kv hd", kv=KVH, hd=HD)

    with tile.TileContext(nc) as tc, ExitStack() as es:
        # ------------------------------------------------------------ constants
        cpool = es.enter_context(tc.tile_pool(name="consts", bufs=1))
        ident = cpool.tile([128, 128], f32)
        make_identity(nc, ident[:])
        ident_bf = cpool.tile([128, 128], bf16)
        nc.vector.tensor_copy(out=ident_bf[:], in_=ident[:])
        eps_t = cpool.tile([128, 1], f32)
        nc.vector.memset(eps_t[:], EPS)
        oeps_t = cpool.tile([128, 1], f32)
        nc.vector.memset(oeps_t[:], float(OUT_SCALE) * EPS)
        lnc_t = cpool.tile([128, 1], f32)
        nc.vector.memset(lnc_t[:], math.log(INV_SQRT_HD))
        # one-hot column tiles: oh_cols[:, h, :] has column h all-ones
        oh_cols = cpool.tile([128, H, H], bf16)
        nc.vector.memset(oh_cols[:], 0.0)
        for h in range(H):
            nc.vector.memset(oh_cols[:, h, h:h + 1], 1.0)
        # one-hot row tiles: ohr[:, 128h:128h+128] has row h all-ones
        ohr = cpool.tile([H, H * 128], bf16)
        nc.sync.dma_start(out=ohr[:], in_=ohr_h[:])
        cosq_sb = cpool.tile([128, 4, HD], f32)
        sinq_sb = cpool.tile([128, 4, HD], f32)
        cosk_sb = cpool.tile([128, 4, HD], f32)
        sink_sb = cpool.tile([128, 4, HD], f32)
        nc.sync.dma_start(out=cosq_sb[:], in_=cos_q.rearrange("(m p) d -> p m d", p=128))
        nc.sync.dma_start(out=sinq_sb[:], in_=sin_q.rearrange("(m p) d -> p m d", p=128))
        nc.sync.dma_start(out=cosk_sb[:], in_=cos_k.rearrange("(m p) d -> p m d", p=128))
        nc.sync.dma_start(out=sink_sb[:], in_=sin_k.rearrange("(m p) d -> p m d", p=128))

        # ============================================================ helpers
        def rms_sumsq(x_t, nh, s2, scrap):
            """s2[:, h] = sum over HD of x_t[:, h*128:...]^2 via the scalar
            engine's Square + row-accumulator (Square lives in every LUT set,
            so no table reload)."""
            for h in range(nh):
                nc.scalar.activation(out=scrap[:], in_=x_t[:, 128 * h:128 * h + 128],
                                     func=AF.Square, accum_out=s2[:, h:h + 1])

        def rms_apply(x_t, nh, ri):
            """x_t *= ri per head (broadcast over HD)."""
            x3 = x_t[:].rearrange("p (h d) -> p h d", h=nh)
            ri_b = bc_free(ri, 128, 2)
            nc.vector.tensor_tensor(out=x3, in0=x3, in1=ri_b, op=ALU.mult)

        def rope_to_bf(dst_bf, src, nh, cos_sb, sin_sb, m, t1, t2):
            """dst_bf bf16 [128, nh*HD] = rope(src f32), ops split between the
            vector (cos mult + lo half) and gpsimd (hi half) engines."""
            half = HD // 2
            d3 = dst_bf[:].rearrange("p (h d) -> p h d", h=nh)
            s3 = src[:].rearrange("p (h d) -> p h d", h=nh)
            cos_b = bc_free(cos_sb[:, m, :], nh, 1)
            sin_lo = bc_free(sin_sb[:, m, 0:half], nh, 1)
            sin_hi = bc_free(sin_sb[:, m, half:HD], nh, 1)
            nc.vector.tensor_tensor(out=d3, in0=s3, in1=cos_b, op=ALU.mult)
            nc.vector.tensor_tensor(out=t1[:], in0=s3[:, :, half:HD],
                                    in1=sin_lo, op=ALU.mult)
            nc.vector.tensor_tensor(out=d3[:, :, 0:half], in0=d3[:, :, 0:half],
                                    in1=t1[:], op=ALU.subtract)
            nc.gpsimd.tensor_tensor(out=t2[:], in0=s3[:, :, 0:half],
                                    in1=sin_hi, op=ALU.mult)
            nc.gpsimd.tensor_tensor(out=d3[:, :, half:HD], in0=d3[:, :, half:HD],
                                    in1=t2[:], op=ALU.add)

        # persistent pools; LIFO such that ppxq (last-opened persistent)
        # can be released after phase 2
        p_gT = es.enter_context(tc.tile_pool(name="ppgT", bufs=1))
        gT_sb = p_gT.tile([128, H, QTOK], bf16, name="gT_sb")
        p_qT = es.enter_context(tc.tile_pool(name="ppqT", bufs=1))
        qT_sb = p_qT.tile([128, H, QTOK], bf16, name="qT_sb")
        p_gTr = es.enter_context(tc.tile_pool(name="ppgTr", bufs=1))
        gTr_sb = p_gTr.tile([128, H, QTOK], bf16, name="gTr_sb")
        xq_es = ExitStack()
        p_xq = xq_es.enter_context(tc.tile_pool(name="ppxq", bufs=1))
        xqT_sb = p_xq.tile([128, 16, QTOK], bf16, name="xqT_sb")

        # ===================================================== phase 1: K / V
        k_stage, v_stage = [], []
        with tc.tile_pool(name="p1xt", bufs=1) as xtp, \
             tc.tile_pool(name="p1w", bufs=1) as wp, \
             tc.tile_pool(name="p1kv", bufs=1) as kvp, \
             tc.tile_pool(name="p1ps", bufs=1, space="PSUM") as pskv, \
             tc.tile_pool(name="p1pt", bufs=2, space="PSUM") as ptp, \
             tc.tile_pool(name="p1sm", bufs=2) as smp:
            xkT_sb = xtp.tile([128, 16, KVTOK + 128], bf16, name="xkT_sb")
            xvT_sb = xtp.tile([128, 16, KVTOK + 128], bf16, name="xvT_sb")
            wk_t = [wp.tile([128, 2 * KVH * HD], bf16, tag=f"w{k}",
                            name=f"wk{k}") for k in range(16)]
            wv_t = [wp.tile([128, 2 * KVH * HD], bf16, tag=f"w{k}",
                            name=f"wv{k}") for k in range(16)]
            # DMA issue order = transfer order: each stream's weights land
            # right after its activations so the first matmuls start early
            nc.sync.dma_start(out=xkT_sb[:],
                              in_=xkT.rearrange("(k p) t -> p k t", p=128))
            for k in range(16):
                nc.sync.dma_start(out=wk_t[k][:], in_=Wkk[128 * k:128 * k + 128, :])
            nc.sync.dma_start(out=xvT_sb[:],
                              in_=xvT.rearrange("(k p) t -> p k t", p=128))
            for k in range(16):
                nc.sync.dma_start(out=wv_t[k][:], in_=Wvv[128 * k:128 * k + 128, :])
            # prefetch xq^T: needed at the top of phase 2, no dependencies
            nc.sync.dma_start(out=xqT_sb[:],
                              in_=xqT.rearrange("(k p) t -> p k t", p=128))
            s2k = kvp.tile([128, 16], f32, name="s2k")
            s2v = kvp.tile([128, 16], f32, name="s2v")
            rik = kvp.tile([128, 16], f32, name="rik")
            riv = kvp.tile([128, 16], f32, name="riv")
            sq_scrap = kvp.tile([128, HD], f32, name="sqsc")
            nat = {}

            pskv_t = {}

            def kv_proj(xT_sb, wts, stg, k0, k1):
                if (stg, 0) not in pskv_t:
                    pskv_t[stg, 0] = [
                        pskv.tile([128, KVH * HD], f32, tag=f"pkv{m}",
                                  name=f"pkv{stg}{m}") for m in range(4)]
                ps = pskv_t[stg, 0]
                for k in range(k0, k1):
                    wt = wts[k]
                    for m in range(4):
                        nc.tensor.matmul(ps[m][:],
                                         xT_sb[:, k, 128 + 128 * m:256 + 128 * m],
                                         wt[:, :KVH * HD], start=(k == 0), stop=False)
                        nc.tensor.matmul(ps[m][:],
                                         xT_sb[:, k, 127 + 128 * m:255 + 128 * m],
                                         wt[:, KVH * HD:], start=False, stop=(k == 15))
                if k1 == 16:
                    for m in range(4):
                        t = kvp.tile([128, KVH * HD], f32, name=f"nat{stg}{m}")
                        nat[stg, m] = t
                        nc.scalar.copy(out=t[:], in_=ps[m][:])

            def rsqrt_batch(s2, ri, bias):
                ln = smp.tile([128, 16], f32, tag="ln")
                nc.scalar.activation(out=ln[:], in_=s2[:], func=AF.Ln,
                                     bias=eps_t[:], scale=1.0 / HD)
                if bias is None:
                    nc.scalar.activation(out=ri, in_=ln[:], func=AF.Exp, scale=-0.5)
                else:
                    nc.scalar.activation(out=ri, in_=ln[:], func=AF.Exp,
                                         scale=-0.5, bias=bias)

            # K projection, K row-sums + rsqrt (scalar runs under V proj)
            kv_proj(xkT_sb, wk_t, "k", 0, 16)
            for m in range(4):
                rms_sumsq(nat["k", m], KVH, s2k[:, 4 * m:4 * m + 4], sq_scrap)
            rsqrt_batch(s2k[:], rik[:], None)
            # first half of the V projection keeps the PE busy under the K
            # postproc chain; K transposes then slot in with zero PE stall
            kv_proj(xvT_sb, wv_t, "v", 0, 8)
            # K scale + rope + transpose + stage -> AllGather(K)
            for m in range(4):
                t = nat["k", m]
                rms_apply(t, KVH, rik[:, 4 * m:4 * m + 4])
                rot_bf = smp.tile([128, KVH * HD], bf16, tag="rotbf")
                t1 = smp.tile([128, KVH, HD // 2], f32, tag="t1")
                t2 = smp.tile([128, KVH, HD // 2], f32, tag="t2")
                rope_to_bf(rot_bf, t, KVH, cosk_sb, sink_sb, m, t1, t2)
                kst = smp.tile([128, KVH, 128], bf16, tag="kst")
                for kv in range(KVH):
                    pst = ptp.tile([128, 128], bf16, tag="pst")
                    nc.tensor.transpose(pst[:], rot_bf[:, 128 * kv:128 * kv + 128],
                                        ident_bf[:])
                    nc.scalar.copy(out=kst[:, kv, :], in_=pst[:])
                d = nc.scalar.dma_start(
                    out=k_loc_T[:, :, 128 * m:128 * m + 128], in_=kst[:])
                k_stage.append(d)
            ag_k = nc.gpsimd.collective_compute(
                "AllGather", ALU.bypass,
                replica_groups=[[0, 1, 2, 3], [4, 5, 6, 7]],
                ins=[k_loc[:]], outs=[k_gath[:]])
            for d in k_stage:
                tile.add_dep_helper(ag_k.ins, d.ins, reason="k stage before ag")
            # second half of the V projection
            kv_proj(xvT_sb, wv_t, "v", 8, 16)
            # V row-sums + rsqrt + scale (writes bf16) + stage -> AllGather(V)
            for m in range(4):
                rms_sumsq(nat["v", m], KVH, s2v[:, 4 * m:4 * m + 4], sq_scrap)
            rsqrt_batch(s2v[:], riv[:], None)
            for m in range(4):
                t = nat["v", m]
                vr = smp.tile([128, KVH * HD], bf16, tag="vr")
                v3 = vr[:].rearrange("p (h d) -> p h d", h=KVH)
                t3 = t[:].rearrange("p (h d) -> p h d", h=KVH)
                ri_b = bc_free(riv[:, 4 * m:4 * m + 4], 128, 2)
                nc.vector.tensor_tensor(out=v3, in0=t3, in1=ri_b, op=ALU.mult)
                d = nc.scalar.dma_start(
                    out=v_loc_v[128 * m:128 * m + 128, :, :],
                    in_=vr[:].rearrange("p (h d) -> p h d", h=KVH))
                v_stage.append(d)
            ag_v = nc.gpsimd.collective_compute(
                "AllGather", ALU.bypass,
                replica_groups=[[0, 1, 2, 3], [4, 5, 6, 7]],
                ins=[v_loc[:]], outs=[v_gath[:]])
            for d in v_stage:
                tile.add_dep_helper(ag_v.ins, d.ins, reason="v stage before ag")

        if "2" not in phases:
            with tc.tile_pool(name="dbg1", bufs=1) as dbp:
                for m in range(4):
                    t = dbp.tile([128, D], f32, tag="dbg")
                    nc.vector.memset(t[:], 0.0)
                    nc.sync.dma_start(out=out_y[128 * m:128 * m + 128, :], in_=t[:])

        # ===================================================== phase 2: Q / G
        if "2" in phases:
          with tc.tile_pool(name="p2w", bufs=1) as wp, \
               tc.tile_pool(name="p2q", bufs=1) as qp, \
               tc.tile_pool(name="p2ps", bufs=1, space="PSUM") as psq, \
               tc.tile_pool(name="p2pt", bufs=2, space="PSUM") as ptp, \
               tc.tile_pool(name="p2sm", bufs=2) as smp:
            # full-row weight tiles: 16 DMA issues per matrix instead of 64
            wq_t = [wp.tile([128, H * HD], bf16, tag=f"w{k}", name=f"wq{k}")
                    for k in range(16)]
            for k in range(16):
                nc.sync.dma_start(out=wq_t[k][:], in_=Wq[128 * k:128 * k + 128, :])

            # Q projection -> natural [tok, H*HD]
            q_sb = [qp.tile([128, H * HD], f32, name=f"q{m}") for m in range(4)]
            for n in range(4):
                ps = [psq.tile([128, 512], f32, tag=f"pp{m}", name=f"pq{m}")
                      for m in range(4)]
                for k in range(16):
                    for m in range(4):
                        nc.tensor.matmul(ps[m][:],
                                         xqT_sb[:, k, 128 * m:128 * m + 128],
                                         wq_t[k][:, 512 * n:512 * n + 512],
                                         start=(k == 0), stop=(k == 15))
                for m in range(4):
                    nc.scalar.copy(out=q_sb[m][:, 512 * n:512 * n + 512], in_=ps[m][:])

            # q row-sums + rsqrt (scale folds 1/sqrt(HD)); runs under G proj
            s2q = qp.tile([128, 4, H], f32, name="s2q")
            riq = qp.tile([128, 4, H], f32, name="riq")
            sq_scrap = qp.tile([128, HD], f32, name="sqscq")
            for m in range(4):
                rms_sumsq(q_sb[m], H, s2q[:, m, :], sq_scrap)
            for m in range(4):
                ln = smp.tile([128, H], f32, tag="qln")
                nc.scalar.activation(out=ln[:], in_=s2q[:, m, :], func=AF.Ln,
                                     bias=eps_t[:], scale=1.0 / HD)
                nc.scalar.activation(out=riq[:, m, :], in_=ln[:], func=AF.Exp,
                                     scale=-0.5, bias=lnc_t[:])

            # G projection -> transposed [gcol, tok] directly, bf16
            wg_t = [wp.tile([128, H * HD], bf16, tag=f"w{k}", name=f"wg{k}")
                    for k in range(16)]
            for k in range(16):
                nc.sync.dma_start(out=wg_t[k][:], in_=Wg[128 * k:128 * k + 128, :])
            for gq in range(4):
                psg = [psq.tile([128, 512], f32, tag=f"pp{i}", name=f"pg{i}")
                       for i in range(4)]
                for k in range(16):
                    for gi in range(4):
                        nc.tensor.matmul(
                            psg[gi][:],
                            wg_t[k][:, 512 * gq + 128 * gi:512 * gq + 128 * gi + 128],
                            xqT_sb[:, k, :],
                            start=(k == 0), stop=(k == 15))
                for gi in range(4):
                    nc.scalar.copy(out=gT_sb[:, 4 * gq + gi, :], in_=psg[gi][:])

            # q scale + rope (under G proj) then transpose
            rots = []
            for m in range(4):
                rms_apply(q_sb[m], H, riq[:, m, :])
                rot_bf = smp.tile([128, H * HD], bf16, tag="qrotbf",
                                  name=f"qrot{m}")
                t1 = smp.tile([128, H, HD // 2], f32, tag="qt1")
                t2 = smp.tile([128, H, HD // 2], f32, tag="qt2")
                rope_to_bf(rot_bf, q_sb[m], H, cosq_sb, sinq_sb, m, t1, t2)
                rots.append(rot_bf)
            for m in range(4):
                for h in range(H):
                    pst = ptp.tile([128, 128], bf16, tag="pst")
                    nc.tensor.transpose(pst[:], rots[m][:, 128 * h:128 * h + 128],
                                        ident_bf[:])
                    nc.scalar.copy(out=qT_sb[:, h, 128 * m:128 * m + 128], in_=pst[:])

        if "2" in phases and "3" not in phases:
            with tc.tile_pool(name="dbg2", bufs=1) as dbp:
                for m in range(4):
                    t = dbp.tile([128, D], f32, tag="dbg")
                    nc.vector.tensor_copy(
                        out=t[:],
                        in_=gT_sb[:, 4 * m:4 * m + 4, :].rearrange("p a b -> p (a b)"))
                    nc.sync.dma_start(out=out_y[128 * m:128 * m + 128, :], in_=t[:])

        xq_es.close()
        # ==================================================== phase 3: attention
        # out-proj weights prefetched during attention so phase 4 never
        # waits on SBUF freed by attention tiles; the loads are issued after
        # the gathered-K/V loads so they don't delay the attention start
        p_wo = es.enter_context(tc.tile_pool(name="ppwo", bufs=1))
        wo_t = [p_wo.tile([128, D], bf16, tag=f"wo{k}", name=f"wo{k}")
                for k in range(16)]
        if "3" in phases:
          with tc.tile_pool(name="p3m", bufs=1) as mp, \
               tc.tile_pool(name="p3kv", bufs=1) as kvp, \
               tc.tile_pool(name="p3pt", bufs=3) as ptq, \
               tc.tile_pool(name="p3y", bufs=2) as yp, \
               tc.tile_pool(name="p3gy", bufs=1) as gyp, \
               tc.tile_pool(name="p3py", bufs=2, space="PSUM") as psy_p, \
               tc.tile_pool(name="p3pn", bufs=1, space="PSUM") as psn_p, \
               tc.tile_pool(name="p3sm", bufs=2) as smp:
            masks_sb = mp.tile([128, NBLK, 2, TB], bf16, name="masks")
            nc.sync.dma_start(
                out=masks_sb[:],
                in_=mask_all.rearrange("p (i s t) -> p i s t", i=NBLK, s=2))

            # gathered K: [128(hd), kv, shard, t] ; V: [128(tok%128), g, kv, hd]
            K_all = kvp.tile([128, KVH, 4, KVTOK], bf16, name="K_all")
            V_all = kvp.tile([128, 16, KVH, HD], bf16, name="V_all")
            for sh in range(4):
                kg = k_gath[sh].rearrange("(kv hd t) -> kv hd t", kv=KVH, hd=HD)
                vg = v_gath[sh].rearrange("(t kv hd) -> t kv hd", kv=KVH, hd=HD)
                d = nc.sync.dma_start(out=K_all[:, :, sh, :],
                                      in_=kg.rearrange("kv d t -> d kv t"))
                tile.add_dep_helper(d.ins, ag_k.ins, reason="ag before k load")
                d = nc.sync.dma_start(
                    out=V_all[:, 4 * sh:4 * sh + 4, :, :],
                    in_=vg.rearrange("(a p) kv d -> p a kv d", p=128))
                tile.add_dep_helper(d.ins, ag_v.ins, reason="ag before v load")
            if "4" in phases:
                for k in range(16):
                    nc.sync.dma_start(out=wo_t[k][:],
                                      in_=Wo[128 * k:128 * k + 128, :])

            gy_sb = gyp.tile([128, H, QTOK], bf16, name="gy_sb")
            n2_ps = psn_p.tile([H, 2 * TB], f32, name="n2")
            # i-order puts full-region AV matmuls at the start and stop flags
            IORD = [0, 4, 5, 6, 7, 1, 2, 3]
            pss_es = ExitStack()
            pss_p = pss_es.enter_context(
                tc.tile_pool(name="p3ps", bufs=2, space="PSUM"))
            for h in range(H):
                kv = h // 4
                psy = psy_p.tile([128, 2 * TB], f32, tag="psy")
                pts = []
                for step in range(len(IORD) + 1):
                    if step < len(IORD):
                        i = IORD[step]
                        big = i < 4
                        if big:
                            pss = pss_p.tile([128, 2, 2 * TB], f32, tag="pss")
                            qs = qT_sb[:, h, :]
                        else:
                            pss = pss_p.tile([128, 2, TB], f32, tag="pss")
                            qs = qT_sb[:, h, TB:2 * TB]
                        for ss in range(2):
                            nc.tensor.matmul(
                                pss[:, ss, :],
                                K_all[:, kv, i // 2,
                                      TB * (i % 2) + 128 * ss:
                                      TB * (i % 2) + 128 * ss + 128],
                                qs, start=True, stop=True)
                        w = 2 * TB if big else TB
                        pt = ptq.tile([128, 2, w], bf16, tag="pt")
                        nc.scalar.activation(
                            out=pt[:].rearrange("p a b -> p (a b)"),
                            in_=pss[:].rearrange("p a b -> p (a b)"), func=AF.Exp)
                        # big tiles: mask only the jq0 half (jq1 half of the
                        # first 4 k-blocks is always past/valid)
                        nc.vector.tensor_tensor(
                            out=pt[:, :, 0:TB], in0=pt[:, :, 0:TB],
                            in1=masks_sb[:, i, :, :], op=ALU.mult)
                        pts.append((i, big, pt))
                    if step >= 1:
                        i, big, pt = pts[step - 1]
                        for ss in range(2):
                            if big:
                                nc.tensor.matmul(
                                    psy[:], V_all[:, 2 * i + ss, kv, :],
                                    pt[:, ss, :],
                                    start=(step == 1 and ss == 0),
                                    stop=(step == len(IORD) and ss == 1))
                            else:
                                nc.tensor.matmul(
                                    psy[:, TB:2 * TB],
                                    V_all[:, 2 * i + ss, kv, :],
                                    pt[:, ss, :], start=False, stop=False)
                ysq = smp.tile([128, 2 * TB], bf16, tag="ysq")
                y_t = yp.tile([128, 2 * TB], bf16, tag="yt")
                nc.vector.tensor_copy(out=y_t[:], in_=psy[:])
                nc.vector.tensor_tensor(out=ysq[:], in0=y_t[:],
                                        in1=y_t[:], op=ALU.mult)
                # gate product early on the otherwise-idle gpsimd engine so
                # the post-attention epilogue is a single multiply per head
                nc.gpsimd.tensor_tensor(out=gy_sb[:, h, :], in0=y_t[:],
                                        in1=gT_sb[:, h, :], op=ALU.mult)
                nc.tensor.matmul(n2_ps[:], oh_cols[:, h, :], ysq[:],
                                 start=(h == 0), stop=(h == H - 1))
            pss_es.close()
            psb_p = pss_es.enter_context(
                tc.tile_pool(name="p3pb", bufs=2, space="PSUM"))
            # epilogue: one Ln+Exp pair for all 32 l2 norms, broadcast + gate
            lnn = smp.tile([H, 2 * TB], f32, tag="lnn")
            nc.scalar.activation(out=lnn[:], in_=n2_ps[:], func=AF.Ln)
            rsq = smp.tile([H, 2 * TB], bf16, tag="rsq")
            nc.scalar.activation(out=rsq[:], in_=lnn[:], func=AF.Exp, scale=-0.5)
            for h in range(H):
                psb = psb_p.tile([128, 2 * TB], f32, tag="psb")
                nc.tensor.matmul(psb[:], ohr[:, 128 * h:128 * h + 128],
                                 rsq[:], start=True, stop=True)
                nc.vector.tensor_tensor(out=gTr_sb[:, h, :],
                                        in0=gy_sb[:, h, :],
                                        in1=psb[:], op=ALU.mult)
            pss_es.close()

        if "3" in phases and "4" not in phases:
            with tc.tile_pool(name="dbg3", bufs=1) as dbp:
                for m in range(4):
                    t = dbp.tile([128, D], f32, tag="dbg")
                    nc.vector.tensor_copy(
                        out=t[:],
                        in_=gTr_sb[:, 4 * m:4 * m + 4, :].rearrange("p a b -> p (a b)"))
                    nc.sync.dma_start(out=out_y[128 * m:128 * m + 128, :], in_=t[:])

        # ==================================================== phase 4: out proj
        if "4" in phases:
          with tc.tile_pool(name="p4o", bufs=2) as op_, \
               tc.tile_pool(name="p4ps", bufs=2, space="PSUM") as pso_p, \
               tc.tile_pool(name="p4sm", bufs=2) as smp:
            if "3" not in phases:
                for k in range(16):
                    nc.sync.dma_start(out=wo_t[k][:],
                                      in_=Wo[128 * k:128 * k + 128, :])
            # pipelined per m-tile: each 128-token tile finishes its matmuls,
            # rms and store while the next tile's matmuls run
            for m in range(4):
                pso = [pso_p.tile([128, 512], f32, tag=f"po{n}", name=f"po{n}")
                       for n in range(4)]
                for k in range(16):
                    for n in range(4):
                        nc.tensor.matmul(pso[n][:],
                                         gTr_sb[:, k, 128 * m:128 * m + 128],
                                         wo_t[k][:, 512 * n:512 * n + 512],
                                         start=(k == 0), stop=(k == 15))
                o_sb = op_.tile([128, D], f32, tag="o", name=f"o{m}")
                s2o = smp.tile([128, 1], f32, tag="s2o")
                sq_sc = smp.tile([128, D], f32, tag="osc")
                for n in range(4):
                    nc.scalar.copy(out=o_sb[:, 512 * n:512 * n + 512],
                                   in_=pso[n][:])
                nc.vector.tensor_tensor(out=sq_sc[:], in0=o_sb[:],
                                        in1=o_sb[:], op=ALU.mult)
                nc.vector.tensor_reduce(out=s2o[:], in_=sq_sc[:],
                                        axis=mybir.AxisListType.X, op=ALU.add)
                lno = smp.tile([128, 1], f32, tag="lno")
                nc.scalar.activation(out=lno[:], in_=s2o[:], func=AF.Ln,
                                     bias=oeps_t[:], scale=float(OUT_SCALE) / D)
                r2o = smp.tile([128, 1], f32, tag="r2o")
                nc.scalar.activation(out=r2o[:], in_=lno[:], func=AF.Exp,
                                     scale=-0.5)
                nc.vector.tensor_scalar_mul(o_sb[:], o_sb[:], r2o[:])
                nc.sync.dma_start(out=out_y[128 * m:128 * m + 128, :],
                                  in_=o_sb[:])

    nc.compile()
    _CACHE[key] = nc
    return nc


def _host_inputs(xq, xk, xv, Wq, Wk, Wv, Wg, Wo, mix_k, mix_v):
    """Build the 8 per-core input maps (bf16 weights/activations)."""
    import ml_dtypes
    f = np.float32
    bf = ml_dtypes.bfloat16
    xq = np.asarray(xq, f)
    xk = np.asarray(xk, f)
    xv = np.asarray(xv, f)
    Wq = np.asarray(Wq, f)
    Wk = np.asarray(Wk, f)
    Wv = np.asarray(Wv, f)
    Wg = np.asarray(Wg, f)
    Wo = np.asarray(Wo, f)
    mix_k = np.asarray(mix_k, f)
    mix_v = np.asarray(mix_v, f)

    Wkk = np.ascontiguousarray(np.concatenate(
        [(1.0 - mix_k)[:, None] * Wk, mix_k[:, None] * Wk], axis=1)).astype(bf)
    Wvv = np.ascontiguousarray(np.concatenate(
        [(1.0 - mix_v)[:, None] * Wv, mix_v[:, None] * Wv], axis=1)).astype(bf)
    Wq_b = np.ascontiguousarray(Wq).astype(bf)
    Wg_b = np.ascontiguousarray(Wg).astype(bf)
    Wo_b = np.ascontiguousarray(Wo).astype(bf)

    half = HD // 2
    inv_freq = 1.0 / (10000.0 ** (np.arange(half, dtype=np.float64) / half))
    ang = np.arange(T, dtype=np.float64)[:, None] * inv_freq[None, :]
    cos_t = np.concatenate([np.cos(ang), np.cos(ang)], axis=-1).astype(f)
    sin_t = np.concatenate([np.sin(ang), np.sin(ang)], axis=-1).astype(f)

    # multiplicative post-exp masks; pt subtile ss holds tk rows
    # 128*ss..128*ss+127 of k-block i; valid iff global tk <= global tq.
    ii = np.arange(128)[:, None]
    jj = np.arange(TB)[None, :]
    diag_mask = np.empty((128, 2, TB), f)
    for ss in range(2):
        diag_mask[:, ss, :] = (128 * ss + ii <= jj).astype(f)
    ones_m = np.ones((128, 2, TB), f)
    zeros_m = np.zeros((128, 2, TB), f)

    def blk_mask(i, jq):
        return diag_mask if i == jq else (ones_m if i < jq else zeros_m)

    ohr_np = np.zeros((H, H * 128), f)
    for h in range(H):
        ohr_np[h, 128 * h:128 * h + 128] = 1.0
    ohr_np = ohr_np.astype(bf)

    in_maps = []
    for c in range(NCORE):
        b, p = divmod(c, 4)
        jq0, jq1 = p, NBLK - 1 - p
        rows_q = np.concatenate([np.arange(TB * jq0, TB * jq0 + TB),
                                 np.arange(TB * jq1, TB * jq1 + TB)])
        t0 = KVTOK * p
        rows_kv = np.arange(t0, t0 + KVTOK)

        xqT_s = np.ascontiguousarray(xq[b, rows_q, :].T.astype(bf))
        xk_s = np.zeros((KVTOK + 128, D), f)
        xv_s = np.zeros((KVTOK + 128, D), f)
        xk_s[128:] = xk[b, t0:t0 + KVTOK, :]
        xv_s[128:] = xv[b, t0:t0 + KVTOK, :]
        if p > 0:
            xk_s[127] = xk[b, t0 - 1, :]
            xv_s[127] = xv[b, t0 - 1, :]
        xkT_s = np.ascontiguousarray(xk_s.T.astype(bf))
        xvT_s = np.ascontiguousarray(xv_s.T.astype(bf))

        # one [128, ss, TB] tile per k-block: jq0 mask for i<4 (the jq1 half
        # of big tiles is always valid), jq1 mask for i>=4
        mask = np.empty((128, NBLK * 2 * TB), f)
        for i in range(NBLK):
            ms = blk_mask(i, jq0 if i < 4 else jq1).reshape(128, 2 * TB)
            mask[:, 2 * TB * i:2 * TB * (i + 1)] = ms

        in_maps.append({
            "xqT": xqT_s, "xkT": xkT_s, "xvT": xvT_s,
            "Wq": Wq_b, "Wg": Wg_b, "Wo": Wo_b,
            "Wkk": Wkk, "Wvv": Wvv,
            "cos_q": np.ascontiguousarray(cos_t[rows_q]),
            "sin_q": np.ascontiguousarray(sin_t[rows_q]),
            "cos_k": np.ascontiguousarray(cos_t[rows_kv]),
            "sin_k": np.ascontiguousarray(sin_t[rows_kv]),
            "mask_all": mask.astype(bf),
            "ohr_h": ohr_np,
        })
    return in_maps


def _run(in_maps, trace=False, tmpdir=None):
    _install_ntff_hook()
    from concourse.bass_utils import run_bass_kernel_spmd
    nc = _build()
    return run_bass_kernel_spmd(nc, in_maps, list(range(NCORE)),
                                trace=trace, tmpdir=tmpdir)


def kernel(xq, xk, xv, Wq, Wk, Wv, Wg, Wo, mix_k, mix_v,
           _trace=False, _tmpdir=None):
    in_maps = _host_inputs(xq, xk, xv, Wq, Wk, Wv, Wg, Wo, mix_k, mix_v)
    res = _run(in_maps, trace=_trace, tmpdir=_tmpdir)
    out = np.empty((B, T, D), np.float32)
    for c in range(NCORE):
        b, p = divmod(c, 4)
        jq0, jq1 = p, NBLK - 1 - p
        y = res.results[c]["out_y"]
        out[b, TB * jq0:TB * jq0 + TB, :] = y[:TB]
        out[b, TB * jq1:TB * jq1 + TB, :] = y[TB:]
    kernel._last_exec_ns = res.exec_time_ns
    return out
